# revision 9
# baseline (speedup 1.0000x reference)
"""AttBlock (GroupNorm -> QKV 1x1conv -> HWxHW attention -> out-proj -> residual)
Trainium2 Bass kernel, 8-core SPMD.

Sharding: core c handles batch n=c//2 and query-half h=c%2. The host permutes
the spatial axis so each core's 2048 queries are always columns [0:2048) of its
input (keys/values use all 4096 columns; attention is permutation-invariant
over keys). Inside a core, flash-style attention streams key-chunks of 128
through PSUM in S^T layout ([keys, queries]) so softmax normalization reduces
over the PSUM partition axis via a ones-matmul, and the attention-weighted
value matmul needs no transposes (v is produced pre-transposed).
"""
import sys
import os

for _p in ("/opt/trn_rl_repo", "/root/.axon_site/_ro/trn_rl_repo"):
    if os.path.isdir(_p) and _p not in sys.path:
        sys.path.insert(0, _p)

import numpy as np
import ml_dtypes
from contextlib import ExitStack

import concourse.bass as bass
import concourse.tile as tile
from concourse import bacc, mybir
from concourse.bass_utils import run_bass_kernel_spmd

F32 = mybir.dt.float32
BF16 = mybir.dt.bfloat16

C = 512            # channels
L = 4096           # H*W
Q = 2048           # queries per core (half the spatial positions)
NCHUNK = C // 128  # 4 channel chunks
NJC = L // 128     # 32 key chunks
NIT = Q // 512     # 4 query tiles of 512
EPS = 1e-5


def _build_nc():
    nc = bacc.Bacc("TRN2", target_bir_lowering=False, debug=False, num_devices=8)

    x_l = nc.dram_tensor("x_local", [C, L], F32, kind="ExternalInput").ap()
    wq_d = nc.dram_tensor("wqT", [C, C], BF16, kind="ExternalInput").ap()
    wk_d = nc.dram_tensor("wkT", [C, C], BF16, kind="ExternalInput").ap()
    wv_d = nc.dram_tensor("wvT", [C, C], BF16, kind="ExternalInput").ap()
    wo_d = nc.dram_tensor("woT", [C, C], BF16, kind="ExternalInput").ap()
    bqs_d = nc.dram_tensor("bq_s", [C], F32, kind="ExternalInput").ap()
    bk_d = nc.dram_tensor("bk", [C], F32, kind="ExternalInput").ap()
    fb_d = nc.dram_tensor("fbias", [C], F32, kind="ExternalInput").ap()
    gsc_d = nc.dram_tensor("gn_scale", [C], F32, kind="ExternalInput").ap()
    gbi_d = nc.dram_tensor("gn_bias", [C], F32, kind="ExternalInput").ap()
    gavg_d = nc.dram_tensor("gavg", [128, 8], F32, kind="ExternalInput").ap()
    gexp_d = nc.dram_tensor("gexp", [8, 128], F32, kind="ExternalInput").ap()
    out_l = nc.dram_tensor("out_local", [C, Q], F32, kind="ExternalOutput").ap()

    x_ch = x_l.rearrange("(c p) l -> c p l", p=128)
    out_ch = out_l.rearrange("(c p) l -> c p l", p=128)

    with tile.TileContext(nc) as tc, ExitStack() as ctx:
        pers = ctx.enter_context(tc.tile_pool(name="pers", bufs=1))
        xpool = ctx.enter_context(tc.tile_pool(name="xpool", bufs=2))
        small = ctx.enter_context(tc.tile_pool(name="small", bufs=2))
        epool = ctx.enter_context(tc.tile_pool(name="epool", bufs=6))
        misc = ctx.enter_context(tc.tile_pool(name="misc", bufs=2))
        psum = ctx.enter_context(tc.tile_pool(name="psum", bufs=8, space="PSUM"))

        # ---- constants / weights into SBUF ----
        def load_w(dram, name):
            ws = []
            d = dram.rearrange("(c p) d -> c p d", p=128)
            for ci in range(NCHUNK):
                t = pers.tile([128, C], BF16, tag=f"{name}{ci}", name=f"{name}{ci}")
                nc.sync.dma_start(t[:], d[ci])
                ws.append(t)
            return ws

        wq_sb = load_w(wq_d, "wq")
        wk_sb = load_w(wk_d, "wk")
        wv_sb = load_w(wv_d, "wv")
        wo_sb = load_w(wo_d, "wo")

        def load_p(dram, name):
            t = pers.tile([128, NCHUNK], F32, tag=name, name=name)
            nc.sync.dma_start(t[:], dram.rearrange("(c p) -> p c", p=128))
            return t

        bqs_sb = load_p(bqs_d, "bqs")
        bk_sb = load_p(bk_d, "bk")
        fb_sb = load_p(fb_d, "fb")
        gsc_sb = load_p(gsc_d, "gsc")
        gbi_sb = load_p(gbi_d, "gbi")

        gavg_sb = pers.tile([128, 8], F32, tag="gavg")
        nc.sync.dma_start(gavg_sb[:], gavg_d)
        gexp_sb = pers.tile([8, 128], F32, tag="gexp")
        nc.sync.dma_start(gexp_sb[:], gexp_d)

        ones_bf = pers.tile([128, 1], BF16, tag="ones")
        nc.vector.memset(ones_bf[:], 1.0)
        eps_sb = pers.tile([128, 1], F32, tag="eps")
        nc.vector.memset(eps_sb[:], EPS)

        # ---- GroupNorm -> h (bf16) ----
        h = []
        for cc in range(NCHUNK):
            xt = xpool.tile([128, L], F32, tag="x")
            nc.sync.dma_start(xt[:], x_ch[cc])

            stats = small.tile([128, 8, 6], F32, tag="stats")
            for sb in range(8):
                nc.vector.bn_stats(out=stats[:, sb, :], in_=xt[:, sb * 512:(sb + 1) * 512])
            mv = small.tile([128, 2], F32, tag="mv")
            nc.vector.bn_aggr(out=mv[:], in_=stats[:])

            # [mean, E[x^2]] per channel
            mv2 = small.tile([128, 2], F32, tag="mv2")
            nc.vector.tensor_mul(mv2[:, 1:2], mv[:, 0:1], mv[:, 0:1])
            nc.vector.tensor_add(mv2[:, 1:2], mv2[:, 1:2], mv[:, 1:2])
            nc.vector.tensor_copy(mv2[:, 0:1], mv[:, 0:1])

            gp = psum.tile([8, 2], F32, tag="bank")
            nc.tensor.matmul(gp[:], gavg_sb[:], mv2[:], start=True, stop=True)
            gs = small.tile([8, 2], F32, tag="gs")
            nc.vector.tensor_copy(gs[:], gp[:])

            # group rstd
            gvar = small.tile([8, 1], F32, tag="gvar")
            nc.vector.tensor_mul(gvar[:], gs[:, 0:1], gs[:, 0:1])
            nc.vector.tensor_sub(gvar[:], gs[:, 1:2], gvar[:])
            gsd = small.tile([8, 1], F32, tag="gsd")
            nc.scalar.activation(out=gsd[:], in_=gvar[:],
                                 func=mybir.ActivationFunctionType.Sqrt,
                                 bias=eps_sb[0:8], scale=1.0)
            grstd = small.tile([8, 1], F32, tag="grstd")
            nc.vector.reciprocal(grstd[:], gsd[:])

            pk = small.tile([8, 2], F32, tag="pk")
            nc.vector.tensor_copy(pk[:, 0:1], gs[:, 0:1])
            nc.vector.tensor_copy(pk[:, 1:2], grstd[:])

            ep = psum.tile([128, 2], F32, tag="bank")
            nc.tensor.matmul(ep[:], gexp_sb[:], pk[:], start=True, stop=True)
            chs = small.tile([128, 2], F32, tag="chs")
            nc.vector.tensor_copy(chs[:], ep[:])

            # per-channel mul/add: h = (x - mean)*rstd*scale + bias
            mulc = small.tile([128, 1], F32, tag="mulc")
            nc.vector.tensor_mul(mulc[:], chs[:, 1:2], gsc_sb[:, cc:cc + 1])
            addc = small.tile([128, 1], F32, tag="addc")
            nc.vector.tensor_mul(addc[:], chs[:, 0:1], mulc[:])
            nc.vector.tensor_sub(addc[:], gbi_sb[:, cc:cc + 1], addc[:])

            ht = pers.tile([128, L], BF16, tag=f"h{cc}")
            nc.vector.tensor_scalar(out=ht[:], in0=xt[:], scalar1=mulc[:],
                                    scalar2=addc[:], op0=mybir.AluOpType.mult,
                                    op1=mybir.AluOpType.add)
            h.append(ht)

        # ---- projections ----
        # k[co][:, j] (bf16), q[co][:, i], vT[j, co] (bf16, pre-transposed)
        k = [pers.tile([128, L], BF16, tag=f"k{cc}", name=f"k{cc}") for cc in range(NCHUNK)]
        for cc in range(NCHUNK):
            for jt in range(L // 512):
                kp = psum.tile([128, 512], F32, tag="bank")
                for ci in range(NCHUNK):
                    nc.tensor.matmul(kp[:], wk_sb[ci][:, cc * 128:(cc + 1) * 128],
                                     h[ci][:, jt * 512:(jt + 1) * 512],
                                     start=(ci == 0), stop=(ci == NCHUNK - 1))
                nc.scalar.activation(out=k[cc][:, jt * 512:(jt + 1) * 512], in_=kp[:],
                                     func=mybir.ActivationFunctionType.Identity,
                                     bias=bk_sb[:, cc:cc + 1], scale=1.0)

        vT = pers.tile([128, NJC, C], BF16, tag="vT")
        for jc in range(NJC):
            vp = psum.tile([128, 512], F32, tag="bank")
            for ci in range(NCHUNK):
                nc.tensor.matmul(vp[:], h[ci][:, jc * 128:(jc + 1) * 128], wv_sb[ci][:],
                                 start=(ci == 0), stop=(ci == NCHUNK - 1))
            nc.scalar.activation(out=vT[:, jc, :], in_=vp[:],
                                 func=mybir.ActivationFunctionType.Copy)

        q = [pers.tile([128, Q], BF16, tag=f"q{cc}", name=f"q{cc}") for cc in range(NCHUNK)]
        for cc in range(NCHUNK):
            for it in range(NIT):
                qp = psum.tile([128, 512], F32, tag="bank")
                for ci in range(NCHUNK):
                    nc.tensor.matmul(qp[:], wq_sb[ci][:, cc * 128:(cc + 1) * 128],
                                     h[ci][:, it * 512:(it + 1) * 512],
                                     start=(ci == 0), stop=(ci == NCHUNK - 1))
                nc.scalar.activation(out=q[cc][:, it * 512:(it + 1) * 512], in_=qp[:],
                                     func=mybir.ActivationFunctionType.Identity,
                                     bias=bqs_sb[:, cc:cc + 1], scale=1.0)

        # ---- attention ----
        for it in range(NIT):
            isl = slice(it * 512, (it + 1) * 512)
            attout = [psum.tile([128, 512], F32, tag="bank", name=f"attout{it}_{co}") for co in range(NCHUNK)]
            csum = psum.tile([1, 512], F32, tag="bank")

            for jc in range(NJC):
                sp = psum.tile([128, 512], F32, tag="bank")
                for ci in range(NCHUNK):
                    nc.tensor.matmul(sp[:], k[ci][:, jc * 128:(jc + 1) * 128],
                                     q[ci][:, isl],
                                     start=(ci == 0), stop=(ci == NCHUNK - 1))
                e = epool.tile([128, 512], BF16, tag="e")
                nc.scalar.activation(out=e[:], in_=sp[:],
                                     func=mybir.ActivationFunctionType.Exp)
                for co in range(NCHUNK):
                    nc.tensor.matmul(attout[co][:], vT[:, jc, co * 128:(co + 1) * 128],
                                     e[:], start=(jc == 0), stop=(jc == NJC - 1))
                nc.tensor.matmul(csum[:], ones_bf[:], e[:],
                                 start=(jc == 0), stop=(jc == NJC - 1))

            recip = misc.tile([1, 512], F32, tag="recip", bufs=1)
            nc.vector.reciprocal_approx_fast(out=recip[:], in_=csum[:])
            bc = misc.tile([128, 512], F32, tag="bc")
            nc.gpsimd.partition_broadcast(bc[:], recip[:])

            attn = []
            for co in range(NCHUNK):
                at = misc.tile([128, 512], BF16, tag=f"attn{co}", name=f"attn{co}", bufs=1)
                nc.vector.tensor_mul(at[:], attout[co][:], bc[:])
                attn.append(at)

            for co in range(NCHUNK):
                op = psum.tile([128, 512], F32, tag="bank")
                for ci in range(NCHUNK):
                    nc.tensor.matmul(op[:], wo_sb[ci][:, co * 128:(co + 1) * 128],
                                     attn[ci][:],
                                     start=(ci == 0), stop=(ci == NCHUNK - 1))
                xres = misc.tile([128, 512], F32, tag="xres")
                nc.sync.dma_start(xres[:], x_ch[co][:, isl])
                ot = misc.tile([128, 512], F32, tag="ot")
                nc.scalar.activation(out=ot[:], in_=op[:],
                                     func=mybir.ActivationFunctionType.Identity,
                                     bias=fb_sb[:, co:co + 1], scale=1.0)
                nc.vector.tensor_add(ot[:], ot[:], xres[:])
                nc.sync.dma_start(out_ch[co][:, isl], ot[:])

    nc.compile()
    return nc


_NC_CACHE = None


def _get_nc():
    global _NC_CACHE
    if _NC_CACHE is None:
        _NC_CACHE = _build_nc()
    return _NC_CACHE


def kernel(x, gn_scale, gn_bias, wq, bq, wk, bk, wv, bv, wo, bo):
    x = np.asarray(x, dtype=np.float32)
    gn_scale = np.asarray(gn_scale, dtype=np.float32)
    gn_bias = np.asarray(gn_bias, dtype=np.float32)
    wq = np.asarray(wq, dtype=np.float32)
    bq = np.asarray(bq, dtype=np.float32)
    wk = np.asarray(wk, dtype=np.float32)
    bk = np.asarray(bk, dtype=np.float32)
    wv = np.asarray(wv, dtype=np.float32)
    bv = np.asarray(bv, dtype=np.float32)
    wo = np.asarray(wo, dtype=np.float32)
    bo = np.asarray(bo, dtype=np.float32)

    N, Cx, H, W = x.shape
    assert (N, Cx, H * W) == (4, C, L)

    s = float(C) ** -0.5
    bf = ml_dtypes.bfloat16
    shared = {
        "wqT": np.ascontiguousarray((wq.T * s).astype(bf)),
        "wkT": np.ascontiguousarray(wk.T.astype(bf)),
        "wvT": np.ascontiguousarray(wv.T.astype(bf)),
        "woT": np.ascontiguousarray(wo.T.astype(bf)),
        "bq_s": (bq * s).astype(np.float32),
        "bk": bk,
        "fbias": (bo + wo.astype(np.float64) @ bv.astype(np.float64)).astype(np.float32),
        "gn_scale": gn_scale,
        "gn_bias": gn_bias,
        "gavg": np.repeat(np.eye(8, dtype=np.float32) / 16.0, 16, axis=0),
        "gexp": np.repeat(np.eye(8, dtype=np.float32), 16, axis=1),
    }

    xf = x.reshape(N, C, L)
    in_maps = []
    for c in range(8):
        n, half = c // 2, c % 2
        xn = xf[n]
        if half == 1:
            xn = np.concatenate([xn[:, Q:], xn[:, :Q]], axis=1)
        in_maps.append({"x_local": np.ascontiguousarray(xn), **shared})

    nc = _get_nc()
    res = run_bass_kernel_spmd(nc, in_maps, core_ids=list(range(8))).results

    out = np.empty((N, C, L), dtype=np.float32)
    for c in range(8):
        n, half = c // 2, c % 2
        out[n, :, half * Q:(half + 1) * Q] = res[c]["out_local"]
    return out.reshape(N, C, H, W)


# revision 11
# speedup vs baseline: 1.2018x; 1.2018x over previous
"""AttBlock (GroupNorm -> QKV 1x1conv -> HWxHW attention -> out-proj -> residual)
Trainium2 Bass kernel, 8-core SPMD.

Sharding: core c handles batch n=c//2 and query-half h=c%2. The host permutes
the spatial axis so each core's 2048 queries are always columns [0:2048) of its
input (keys/values use all 4096 columns; attention is permutation-invariant
over keys). Inside a core, flash-style attention streams key-chunks of 128
through PSUM in S^T layout ([keys, queries]) so softmax normalization reduces
over the PSUM partition axis via a ones-matmul, and the attention-weighted
value matmul needs no transposes (v is produced pre-transposed).
"""
import sys
import os

for _p in ("/opt/trn_rl_repo", "/root/.axon_site/_ro/trn_rl_repo"):
    if os.path.isdir(_p) and _p not in sys.path:
        sys.path.insert(0, _p)

import numpy as np
import ml_dtypes
from contextlib import ExitStack

import concourse.bass as bass
import concourse.tile as tile
from concourse import bacc, mybir
from concourse.bass_utils import run_bass_kernel_spmd

F32 = mybir.dt.float32
BF16 = mybir.dt.bfloat16

C = 512            # channels
L = 4096           # H*W
Q = 2048           # queries per core (half the spatial positions)
NCHUNK = C // 128  # 4 channel chunks
NJC = L // 128     # 32 key chunks
NIT = Q // 512     # 4 query tiles of 512
EPS = 1e-5


def _build_nc():
    nc = bacc.Bacc("TRN2", target_bir_lowering=False, debug=False, num_devices=8)

    x_l = nc.dram_tensor("x_local", [C, L], F32, kind="ExternalInput").ap()
    wq_d = nc.dram_tensor("wqT", [C, C], BF16, kind="ExternalInput").ap()
    wk_d = nc.dram_tensor("wkT", [C, C], BF16, kind="ExternalInput").ap()
    wv_d = nc.dram_tensor("wvT", [C, C], BF16, kind="ExternalInput").ap()
    wo_d = nc.dram_tensor("woT", [C, C], BF16, kind="ExternalInput").ap()
    bqs_d = nc.dram_tensor("bq_s", [C], F32, kind="ExternalInput").ap()
    bk_d = nc.dram_tensor("bk", [C], F32, kind="ExternalInput").ap()
    fb_d = nc.dram_tensor("fbias", [C], F32, kind="ExternalInput").ap()
    gsc_d = nc.dram_tensor("gn_scale", [C], F32, kind="ExternalInput").ap()
    gbi_d = nc.dram_tensor("gn_bias", [C], F32, kind="ExternalInput").ap()
    gavg_d = nc.dram_tensor("gavg", [128, 8], F32, kind="ExternalInput").ap()
    gexp_d = nc.dram_tensor("gexp", [8, 128], F32, kind="ExternalInput").ap()
    out_l = nc.dram_tensor("out_local", [C, Q], F32, kind="ExternalOutput").ap()

    x_ch = x_l.rearrange("(c p) l -> c p l", p=128)
    out_ch = out_l.rearrange("(c p) l -> c p l", p=128)

    with tile.TileContext(nc) as tc, ExitStack() as ctx:
        pers = ctx.enter_context(tc.tile_pool(name="pers", bufs=1))
        xpool = ctx.enter_context(tc.tile_pool(name="xpool", bufs=2))
        small = ctx.enter_context(tc.tile_pool(name="small", bufs=2))
        epool = ctx.enter_context(tc.tile_pool(name="epool", bufs=8))
        misc = ctx.enter_context(tc.tile_pool(name="misc", bufs=2))
        psum = ctx.enter_context(tc.tile_pool(name="psum", bufs=8, space="PSUM"))

        # ---- constants / weights into SBUF ----
        def load_w(dram, name):
            ws = []
            d = dram.rearrange("(c p) d -> c p d", p=128)
            for ci in range(NCHUNK):
                t = pers.tile([128, C], BF16, tag=f"{name}{ci}", name=f"{name}{ci}")
                nc.sync.dma_start(t[:], d[ci])
                ws.append(t)
            return ws

        wq_sb = load_w(wq_d, "wq")
        wk_sb = load_w(wk_d, "wk")
        wv_sb = load_w(wv_d, "wv")
        wo_sb = load_w(wo_d, "wo")

        def load_p(dram, name):
            t = pers.tile([128, NCHUNK], F32, tag=name, name=name)
            nc.sync.dma_start(t[:], dram.rearrange("(c p) -> p c", p=128))
            return t

        bqs_sb = load_p(bqs_d, "bqs")
        bk_sb = load_p(bk_d, "bk")
        fb_sb = load_p(fb_d, "fb")
        gsc_sb = load_p(gsc_d, "gsc")
        gbi_sb = load_p(gbi_d, "gbi")

        gavg_sb = pers.tile([128, 8], F32, tag="gavg")
        nc.sync.dma_start(gavg_sb[:], gavg_d)
        gexp_sb = pers.tile([8, 128], F32, tag="gexp")
        nc.sync.dma_start(gexp_sb[:], gexp_d)

        ones_f32 = pers.tile([128, 1], F32, tag="ones_f32")
        nc.vector.memset(ones_f32[:], 1.0)
        eps_sb = pers.tile([128, 1], F32, tag="eps")
        nc.vector.memset(eps_sb[:], EPS)

        # ---- GroupNorm -> h (bf16) ----
        h = []
        for cc in range(NCHUNK):
            xt = xpool.tile([128, L], F32, tag="x")
            nc.sync.dma_start(xt[:], x_ch[cc])

            stats = small.tile([128, 8, 6], F32, tag="stats")
            for sb in range(8):
                nc.vector.bn_stats(out=stats[:, sb, :], in_=xt[:, sb * 512:(sb + 1) * 512])
            mv = small.tile([128, 2], F32, tag="mv")
            nc.vector.bn_aggr(out=mv[:], in_=stats[:])

            # [mean, E[x^2]] per channel
            mv2 = small.tile([128, 2], F32, tag="mv2")
            nc.vector.tensor_mul(mv2[:, 1:2], mv[:, 0:1], mv[:, 0:1])
            nc.vector.tensor_add(mv2[:, 1:2], mv2[:, 1:2], mv[:, 1:2])
            nc.vector.tensor_copy(mv2[:, 0:1], mv[:, 0:1])

            gp = psum.tile([8, 2], F32, tag="bank")
            nc.tensor.matmul(gp[:], gavg_sb[:], mv2[:], start=True, stop=True)
            gs = small.tile([8, 2], F32, tag="gs")
            nc.vector.tensor_copy(gs[:], gp[:])

            # group rstd
            gvar = small.tile([8, 1], F32, tag="gvar")
            nc.vector.tensor_mul(gvar[:], gs[:, 0:1], gs[:, 0:1])
            nc.vector.tensor_sub(gvar[:], gs[:, 1:2], gvar[:])
            gsd = small.tile([8, 1], F32, tag="gsd")
            nc.scalar.activation(out=gsd[:], in_=gvar[:],
                                 func=mybir.ActivationFunctionType.Sqrt,
                                 bias=eps_sb[0:8], scale=1.0)
            grstd = small.tile([8, 1], F32, tag="grstd")
            nc.vector.reciprocal(grstd[:], gsd[:])

            pk = small.tile([8, 2], F32, tag="pk")
            nc.vector.tensor_copy(pk[:, 0:1], gs[:, 0:1])
            nc.vector.tensor_copy(pk[:, 1:2], grstd[:])

            ep = psum.tile([128, 2], F32, tag="bank")
            nc.tensor.matmul(ep[:], gexp_sb[:], pk[:], start=True, stop=True)
            chs = small.tile([128, 2], F32, tag="chs")
            nc.vector.tensor_copy(chs[:], ep[:])

            # per-channel mul/add: h = (x - mean)*rstd*scale + bias
            mulc = small.tile([128, 1], F32, tag="mulc")
            nc.vector.tensor_mul(mulc[:], chs[:, 1:2], gsc_sb[:, cc:cc + 1])
            addc = small.tile([128, 1], F32, tag="addc")
            nc.vector.tensor_mul(addc[:], chs[:, 0:1], mulc[:])
            nc.vector.tensor_sub(addc[:], gbi_sb[:, cc:cc + 1], addc[:])

            ht = pers.tile([128, L], BF16, tag=f"h{cc}")
            nc.vector.tensor_scalar(out=ht[:], in0=xt[:], scalar1=mulc[:],
                                    scalar2=addc[:], op0=mybir.AluOpType.mult,
                                    op1=mybir.AluOpType.add)
            h.append(ht)

        # ---- projections ----
        # k[co][:, j] (bf16), q[co][:, i], vT[j, co] (bf16, pre-transposed)
        k = [pers.tile([128, L], BF16, tag=f"k{cc}", name=f"k{cc}") for cc in range(NCHUNK)]
        for cc in range(NCHUNK):
            for jt in range(L // 512):
                kp = psum.tile([128, 512], F32, tag="bank")
                for ci in range(NCHUNK):
                    nc.tensor.matmul(kp[:], wk_sb[ci][:, cc * 128:(cc + 1) * 128],
                                     h[ci][:, jt * 512:(jt + 1) * 512],
                                     start=(ci == 0), stop=(ci == NCHUNK - 1))
                nc.scalar.activation(out=k[cc][:, jt * 512:(jt + 1) * 512], in_=kp[:],
                                     func=mybir.ActivationFunctionType.Identity,
                                     bias=bk_sb[:, cc:cc + 1], scale=1.0)

        vT = pers.tile([128, NJC, C], BF16, tag="vT")
        for jc in range(NJC):
            vp = psum.tile([128, 512], F32, tag="bank")
            for ci in range(NCHUNK):
                nc.tensor.matmul(vp[:], h[ci][:, jc * 128:(jc + 1) * 128], wv_sb[ci][:],
                                 start=(ci == 0), stop=(ci == NCHUNK - 1))
            nc.scalar.activation(out=vT[:, jc, :], in_=vp[:],
                                 func=mybir.ActivationFunctionType.Copy)

        q = [pers.tile([128, Q], BF16, tag=f"q{cc}", name=f"q{cc}") for cc in range(NCHUNK)]
        for cc in range(NCHUNK):
            for it in range(NIT):
                qp = psum.tile([128, 512], F32, tag="bank")
                for ci in range(NCHUNK):
                    nc.tensor.matmul(qp[:], wq_sb[ci][:, cc * 128:(cc + 1) * 128],
                                     h[ci][:, it * 512:(it + 1) * 512],
                                     start=(ci == 0), stop=(ci == NCHUNK - 1))
                nc.scalar.activation(out=q[cc][:, it * 512:(it + 1) * 512], in_=qp[:],
                                     func=mybir.ActivationFunctionType.Identity,
                                     bias=bqs_sb[:, cc:cc + 1], scale=1.0)

        # ---- attention ----
        # Software-pipelined: within the j-loop, S^T runs D chunks ahead of AV
        # so PSUM-slot waits never stall the in-order PE queue; the o-projection
        # of tile t is emitted inside tile t+1's j-loop shadow.
        D = 6  # S^T lookahead depth (epool bufs must be >= D+2)

        def emit_oproj(attn, it):
            isl = slice(it * 512, (it + 1) * 512)
            for co in range(NCHUNK):
                op = psum.tile([128, 512], F32, tag="bank", name=f"op{it}_{co}")
                for ci in range(NCHUNK):
                    nc.tensor.matmul(op[:], wo_sb[ci][:, co * 128:(co + 1) * 128],
                                     attn[ci][:],
                                     start=(ci == 0), stop=(ci == NCHUNK - 1))
                xres = misc.tile([128, 512], F32, tag="xres")
                nc.sync.dma_start(xres[:], x_ch[co][:, isl])
                ot = misc.tile([128, 512], F32, tag="ot")
                nc.scalar.activation(out=ot[:], in_=op[:],
                                     func=mybir.ActivationFunctionType.Identity,
                                     bias=fb_sb[:, co:co + 1], scale=1.0)
                nc.vector.tensor_add(ot[:], ot[:], xres[:])
                nc.sync.dma_start(out_ch[co][:, isl], ot[:])

        pend = None
        for it in range(NIT):
            isl = slice(it * 512, (it + 1) * 512)
            attout = [psum.tile([128, 512], F32, tag="bank", name=f"attout{it}_{co}")
                      for co in range(NCHUNK)]
            esum = misc.tile([128, 512], F32, tag="esum")

            es = []  # staged (e tile, jc)
            for pos in range(NJC + D):
                if pos < NJC:
                    jc = pos
                    sp = psum.tile([128, 512], F32, tag="bank", name="sp")
                    for ci in range(NCHUNK):
                        nc.tensor.matmul(sp[:], k[ci][:, jc * 128:(jc + 1) * 128],
                                         q[ci][:, isl],
                                         start=(ci == 0), stop=(ci == NCHUNK - 1))
                    e = epool.tile([128, 512], BF16, tag="e")
                    nc.scalar.activation(out=e[:], in_=sp[:],
                                         func=mybir.ActivationFunctionType.Exp)
                    if jc == 0:
                        nc.vector.tensor_copy(esum[:], e[:])
                    else:
                        nc.vector.tensor_add(esum[:], esum[:], e[:])
                    es.append((e, jc))
                if pos >= D:
                    e, jc = es[pos - D]
                    for co in range(NCHUNK):
                        nc.tensor.matmul(attout[co][:],
                                         vT[:, jc, co * 128:(co + 1) * 128],
                                         e[:], start=(jc == 0), stop=(jc == NJC - 1))
                if pos == D - 1 and pend is not None:
                    # previous tile's o-projection: slots into the pipeline
                    # while this tile's S^T stream keeps PE busy
                    emit_oproj(*pend)
                    pend = None

            csum = psum.tile([1, 512], F32, tag="bank")
            nc.tensor.matmul(csum[:], ones_f32[:], esum[:], start=True, stop=True)
            recip = misc.tile([1, 512], F32, tag="recip", bufs=1)
            nc.vector.reciprocal_approx_fast(out=recip[:], in_=csum[:])
            bc = misc.tile([128, 512], F32, tag="bc")
            nc.gpsimd.partition_broadcast(bc[:], recip[:])

            attn = []
            for co in range(NCHUNK):
                at = misc.tile([128, 512], BF16, tag=f"attn{co}", name=f"attn{co}")
                nc.vector.tensor_mul(at[:], attout[co][:], bc[:])
                attn.append(at)
            pend = (attn, it)

        emit_oproj(*pend)

    nc.compile()
    return nc


_NC_CACHE = None


def _get_nc():
    global _NC_CACHE
    if _NC_CACHE is None:
        _NC_CACHE = _build_nc()
    return _NC_CACHE


def kernel(x, gn_scale, gn_bias, wq, bq, wk, bk, wv, bv, wo, bo):
    x = np.asarray(x, dtype=np.float32)
    gn_scale = np.asarray(gn_scale, dtype=np.float32)
    gn_bias = np.asarray(gn_bias, dtype=np.float32)
    wq = np.asarray(wq, dtype=np.float32)
    bq = np.asarray(bq, dtype=np.float32)
    wk = np.asarray(wk, dtype=np.float32)
    bk = np.asarray(bk, dtype=np.float32)
    wv = np.asarray(wv, dtype=np.float32)
    bv = np.asarray(bv, dtype=np.float32)
    wo = np.asarray(wo, dtype=np.float32)
    bo = np.asarray(bo, dtype=np.float32)

    N, Cx, H, W = x.shape
    assert (N, Cx, H * W) == (4, C, L)

    s = float(C) ** -0.5
    bf = ml_dtypes.bfloat16
    shared = {
        "wqT": np.ascontiguousarray((wq.T * s).astype(bf)),
        "wkT": np.ascontiguousarray(wk.T.astype(bf)),
        "wvT": np.ascontiguousarray(wv.T.astype(bf)),
        "woT": np.ascontiguousarray(wo.T.astype(bf)),
        "bq_s": (bq * s).astype(np.float32),
        "bk": bk,
        "fbias": (bo + wo.astype(np.float64) @ bv.astype(np.float64)).astype(np.float32),
        "gn_scale": gn_scale,
        "gn_bias": gn_bias,
        "gavg": np.repeat(np.eye(8, dtype=np.float32) / 16.0, 16, axis=0),
        "gexp": np.repeat(np.eye(8, dtype=np.float32), 16, axis=1),
    }

    xf = x.reshape(N, C, L)
    in_maps = []
    for c in range(8):
        n, half = c // 2, c % 2
        xn = xf[n]
        if half == 1:
            xn = np.concatenate([xn[:, Q:], xn[:, :Q]], axis=1)
        in_maps.append({"x_local": np.ascontiguousarray(xn), **shared})

    nc = _get_nc()
    res = run_bass_kernel_spmd(nc, in_maps, core_ids=list(range(8))).results

    out = np.empty((N, C, L), dtype=np.float32)
    for c in range(8):
        n, half = c // 2, c % 2
        out[n, :, half * Q:(half + 1) * Q] = res[c]["out_local"]
    return out.reshape(N, C, H, W)
